# revision 7
# baseline (speedup 1.0000x reference)
"""Trainium2 Bass kernel for AdjStackAttentionWeights.

reference:  out = einsum('bsij,hs->bhij', stacks, W) + b[None,:,None,None]
            out = where(mask[:,None,:,:], 0.0, out)
shapes:     stacks [16,16,512,512] f32, mask [16,512,512] bool,
            W [8,16] f32, b [8] f32  ->  out [16,8,512,512] f32

Data-parallel over batch: 2 graphs per core x 8 cores. Per core, per
16-row i-block q (i = 16q + 2*ih + il, ih in [0,8), il in {0,1}):
  rhs tile [128,1024]: p = 8s+ih, f = il*512+j   (512KB in-DMA, 4KB runs)
  per il: matmul with scattered-block lhsT w_bd[8s+ih, 8ih'+h] =
    W[h,s]*[ih==ih'] -> psum[64il+8ih+h, j] = sum_s W[h,s]*stacks[s,i,j]
    (col-tiled: tile_position (0, 64*il))
  mask broadcast over h via tiny matmul (btn[ih, 8ih'+h] = -[ih==ih'])
  epilogue: ACT keep=1-m ; DVE out = (psum + bias) * keep
  out-DMA per (q,il): [64,512] -> dram dims (ih, h, j)
Matmuls run as float32r (TF32-like) so no input cast is needed.
"""

import numpy as np
import ml_dtypes

B, S, N, H = 16, 16, 512, 8
NCORES = 8
BPC = B // NCORES  # graphs per core

MODE = "f32r"  # "f32r" | "bf16"

_CACHE = {}


def _build():
    import concourse.bacc as bacc
    import concourse.mybir as mybir
    import concourse.tile as tile

    f32 = mybir.dt.float32
    bf16 = mybir.dt.bfloat16
    u8 = mybir.dt.uint8
    cdt = mybir.dt.float32r if MODE == "f32r" else bf16

    nc = bacc.Bacc("TRN2", target_bir_lowering=False, debug=False,
                   num_devices=NCORES)

    stacks = nc.dram_tensor("stacks", [BPC, S, N, N],
                            cdt if MODE == "f32r" else f32,
                            kind="ExternalInput")
    mask = nc.dram_tensor("mask", [BPC, N, N], u8, kind="ExternalInput")
    w_bd = nc.dram_tensor("w_bd", [128, 256], cdt, kind="ExternalInput")
    btn = nc.dram_tensor("btn", [8, 256], bf16, kind="ExternalInput")
    bias = nc.dram_tensor("bias", [128, 1], f32, kind="ExternalInput")
    out = nc.dram_tensor("out", [BPC, H, N, N], f32, kind="ExternalOutput")

    # in-DMA per (b,q): [s(16), ih(8), (il j)(1024)] -> rhs tile [128, 1024]
    sview = stacks.ap().rearrange("b s (q ih il) j -> b q s ih (il j)",
                                  q=32, ih=8, il=2)
    # mask tile per (b,qq): [8, 4096]: p=ih, f = sub*1024 + il*512 + j
    mview = mask.ap().rearrange("b (qq sub ih il) j -> b qq ih sub (il j)",
                                qq=8, sub=4, ih=8, il=2)
    # out-DMA per (b,q,il): [64,512] -> dram dims (ih, h, j)
    oview = out.ap().rearrange("b h (q ih il) j -> b q il ih h j",
                               q=32, ih=8, il=2)

    Ident = mybir.ActivationFunctionType.Identity
    ADD = mybir.AluOpType.add
    MULT = mybir.AluOpType.mult

    with tile.TileContext(nc) as tc:
        with (
            tc.tile_pool(name="const", bufs=1) as cpool,
            tc.tile_pool(name="maskp", bufs=2) as mpool,
            tc.tile_pool(name="data", bufs=4) as dpool,
            tc.tile_pool(name="keep", bufs=3) as kpool,
            tc.tile_pool(name="outp", bufs=4) as opool,
            tc.tile_pool(name="psd", bufs=3, space="PSUM") as psd_pool,
            tc.tile_pool(name="psk", bufs=3, space="PSUM") as psk_pool,
        ):
            wbd_t = cpool.tile([128, 256], cdt)
            nc.sync.dma_start(wbd_t[:], w_bd.ap())
            btn_t = cpool.tile([8, 256], bf16)
            nc.sync.dma_start(btn_t[:], btn.ap())
            bias_t = cpool.tile([128, 1], f32)
            nc.sync.dma_start(bias_t[:], bias.ap())

            for bb in range(BPC):
                for qq in range(8):
                    mask_t = mpool.tile([8, 4096], bf16)
                    nc.gpsimd.dma_start(mask_t[:], mview[bb, qq])  # u8->bf16
                    for sub in range(4):
                        q = qq * 4 + sub
                        rhs_t = dpool.tile([128, 1024], cdt)
                        if MODE == "f32r":
                            nc.sync.dma_start(rhs_t[:], sview[bb, q])
                        else:
                            nc.gpsimd.dma_start(rhs_t[:], sview[bb, q])
                        ps_d = psd_pool.tile([128, 512], f32)
                        ps_k = psk_pool.tile([128, 512], f32)
                        for il in range(2):
                            # lhsT halves are zero except columns
                            # [64il, 64il+64); the two matmuls accumulate
                            # into one [128,512] bank at base partition 0.
                            nc.tensor.matmul(
                                ps_d[:, :], wbd_t[:, il * 128:il * 128 + 128],
                                rhs_t[:, il * 512:il * 512 + 512],
                                start=(il == 0), stop=(il == 1))
                            nc.tensor.matmul(
                                ps_k[:, :], btn_t[:, il * 128:il * 128 + 128],
                                mask_t[:, sub * 1024 + il * 512:
                                       sub * 1024 + il * 512 + 512],
                                start=(il == 0), stop=(il == 1))
                        keep_t = kpool.tile([128, 512], f32)
                        # keep = 1 - m   (ps_k holds -m)
                        nc.scalar.activation(keep_t[:], ps_k[:], Ident,
                                             bias=1.0, scale=1.0)
                        out_t = opool.tile([128, 512], f32)
                        # out = (ps_d + bias) * keep
                        nc.vector.scalar_tensor_tensor(
                            out_t[:], ps_d[:], bias_t[:], keep_t[:],
                            op0=ADD, op1=MULT)
                        for il in range(2):
                            nc.sync.dma_start(oview[bb, q, il],
                                              out_t[64 * il:64 * il + 64, :])

    nc.compile()
    return nc


def _prep_consts(W, b):
    # lhsT for the il-th accumulating matmul lives in w_bd[:, 128*il:...]
    # w_bd[8s+ih, 128*il + 64*il + 8*ih + h] = W[h, s]; rest zero.
    w_bd = np.zeros((128, 256), dtype=np.float32)
    btn = np.zeros((8, 256), dtype=np.float32)
    for il in range(2):
        base = 128 * il + 64 * il
        for ih in range(8):
            for h in range(8):
                w_bd[ih::8, base + 8 * ih + h] = W[h, :]  # rows k = 8s+ih
            btn[ih, base + 8 * ih:base + 8 * ih + 8] = -1.0
    bias = np.tile(np.asarray(b, np.float32), 16).reshape(128, 1)
    if MODE == "bf16":
        w_bd = w_bd.astype(ml_dtypes.bfloat16)
    return w_bd, btn.astype(ml_dtypes.bfloat16), bias


def kernel(stacks, mask, W, b):
    from concourse.bass_utils import run_bass_kernel_spmd

    if "nc" not in _CACHE:
        _CACHE["nc"] = _build()
    nc = _CACHE["nc"]

    stacks = np.ascontiguousarray(np.asarray(stacks, dtype=np.float32))
    mask_u8 = np.ascontiguousarray(np.asarray(mask).view(np.uint8))
    w_bd, btn, bias = _prep_consts(np.asarray(W, np.float32),
                                   np.asarray(b, np.float32))

    in_maps = []
    for c in range(NCORES):
        in_maps.append({
            "stacks": stacks[c * BPC:(c + 1) * BPC],
            "mask": mask_u8[c * BPC:(c + 1) * BPC],
            "w_bd": w_bd, "btn": btn, "bias": bias,
        })

    res = run_bass_kernel_spmd(nc, in_maps, core_ids=list(range(NCORES)),
                               **_CACHE.get("run_kwargs", {}))
    _CACHE["last_result"] = res
    outs = [r["out"] for r in res.results]
    return np.concatenate(outs, axis=0)


# revision 8
# speedup vs baseline: 1.1180x; 1.1180x over previous
"""Trainium2 Bass kernel for AdjStackAttentionWeights.

reference:  out = einsum('bsij,hs->bhij', stacks, W) + b[None,:,None,None]
            out = where(mask[:,None,:,:], 0.0, out)
shapes:     stacks [16,16,512,512] f32, mask [16,512,512] bool,
            W [8,16] f32, b [8] f32  ->  out [16,8,512,512] f32

Data-parallel over batch: 2 graphs per core x 8 cores.

Per graph, i is processed in 4 superblocks w of 128 rows; i decomposes as
i = 128*w + 64*a' + 8*ih + il  (a' in {0,1}, ih in [0,8), il in [0,8)).

  in tile  [128,4096] f32 (one per (b, a=2w+a')): p = 8s+ih, f = il*512+j
           -> 2MB DMA, 16KB contiguous runs
  psum     [128,512] per (w, il): p = 64a' + 8ih + h = 8*cd + h
           (cd = 8a'+ih); two zero-padded-lhsT matmuls accumulate (a'=0,1)
           lhsT w_bd[8s+ih, 128a' + 64a' + 8ih + h] = W[h,s]
  mask     broadcast over h via tiny matmuls (btn -> psum_k = -m)
  epilogue ACT keep=1-m ; DVE out_f = (psum + bias) * keep
  out tile [128,4096] f32 per (b,w): p = 8cd+h, f = il*512+j
           -> 2MB DMA, 16KB contiguous runs
Matmuls run as float32r (TF32-like) so no input cast is needed.
"""

import numpy as np
import ml_dtypes

B, S, N, H = 16, 16, 512, 8
NCORES = 8
BPC = B // NCORES  # graphs per core

MODE = "f32r"  # "f32r" | "bf16"

_CACHE = {}


def _build():
    import concourse.bacc as bacc
    import concourse.mybir as mybir
    import concourse.tile as tile

    f32 = mybir.dt.float32
    bf16 = mybir.dt.bfloat16
    cdt = mybir.dt.float32r if MODE == "f32r" else bf16

    nc = bacc.Bacc("TRN2", target_bir_lowering=False, debug=False,
                   num_devices=NCORES)

    stacks = nc.dram_tensor("stacks", [BPC, S, N, N],
                            cdt if MODE == "f32r" else f32,
                            kind="ExternalInput")
    mask = nc.dram_tensor("mask", [BPC, N, N], bf16, kind="ExternalInput")
    w_bd = nc.dram_tensor("w_bd", [128, 256], cdt, kind="ExternalInput")
    btn = nc.dram_tensor("btn", [8, 256], bf16, kind="ExternalInput")
    bias = nc.dram_tensor("bias", [128, 1], f32, kind="ExternalInput")
    out = nc.dram_tensor("out", [BPC, H, N, N], f32, kind="ExternalOutput")

    # in-DMA per (b, a): [s(16), ih(8), (il j)(4096)] ; i = 64a + 8ih + il
    sview = stacks.ap().rearrange("b s (a ih il) j -> b a s ih (il j)",
                                  a=8, ih=8, il=8)
    # mask per (b, w, ap): [ih(8), (il j)(4096)] ; i = 128w + 64ap + 8ih + il
    mview = mask.ap().rearrange("b (w ap ih il) j -> b w ap ih (il j)",
                                w=4, ap=2, ih=8, il=8)
    # out per (b, w): [cd(16), h(8), (il j)(4096)] ; i = 128w + 8cd + il
    oview = out.ap().rearrange("b h (w cd il) j -> b w cd h (il j)",
                               w=4, cd=16, il=8)

    Ident = mybir.ActivationFunctionType.Identity
    ADD = mybir.AluOpType.add
    MULT = mybir.AluOpType.mult

    with tile.TileContext(nc) as tc:
        with (
            tc.tile_pool(name="const", bufs=1) as cpool,
            tc.tile_pool(name="maskp", bufs=4) as mpool,
            tc.tile_pool(name="data", bufs=4) as dpool,
            tc.tile_pool(name="keep", bufs=3) as kpool,
            tc.tile_pool(name="outp", bufs=2) as opool,
            tc.tile_pool(name="psd", bufs=3, space="PSUM") as psd_pool,
            tc.tile_pool(name="psk", bufs=3, space="PSUM") as psk_pool,
        ):
            wbd_t = cpool.tile([128, 256], cdt)
            nc.sync.dma_start(wbd_t[:], w_bd.ap())
            btn_t = cpool.tile([8, 256], bf16)
            nc.sync.dma_start(btn_t[:], btn.ap())
            bias_t = cpool.tile([128, 1], f32)
            nc.sync.dma_start(bias_t[:], bias.ap())

            for bb in range(BPC):
                for w in range(4):
                    rhs_t = []
                    mask_t = []
                    for ap in range(2):
                        r = dpool.tile([128, 4096], cdt, tag="rhs")
                        nc.sync.dma_start(r[:], sview[bb, 2 * w + ap])
                        rhs_t.append(r)
                        m = mpool.tile([8, 4096], bf16, tag="mask")
                        nc.scalar.dma_start(m[:], mview[bb, w, ap])
                        mask_t.append(m)
                    out_t = opool.tile([128, 4096], f32)
                    for il in range(8):
                        ps_d = psd_pool.tile([128, 512], f32)
                        ps_k = psk_pool.tile([128, 512], f32)
                        for ap in range(2):
                            nc.tensor.matmul(
                                ps_d[:, :],
                                wbd_t[:, ap * 128:ap * 128 + 128],
                                rhs_t[ap][:, il * 512:il * 512 + 512],
                                start=(ap == 0), stop=(ap == 1))
                            nc.tensor.matmul(
                                ps_k[:, :],
                                btn_t[:, ap * 128:ap * 128 + 128],
                                mask_t[ap][:, il * 512:il * 512 + 512],
                                start=(ap == 0), stop=(ap == 1))
                        keep_t = kpool.tile([128, 512], f32)
                        # keep = 1 - m   (ps_k holds -m)
                        nc.scalar.activation(keep_t[:], ps_k[:], Ident,
                                             bias=1.0, scale=1.0)
                        # out = (ps_d + bias) * keep
                        nc.vector.scalar_tensor_tensor(
                            out_t[:, il * 512:il * 512 + 512], ps_d[:],
                            bias_t[:], keep_t[:], op0=ADD, op1=MULT)
                    eng = nc.sync if (bb * 4 + w) % 2 == 0 else nc.scalar
                    eng.dma_start(oview[bb, w], out_t[:])

    nc.compile()
    return nc


def _prep_consts(W, b):
    # lhsT for the ap-th accumulating matmul lives in w_bd[:, 128*ap:...]
    # w_bd[8s+ih, 128*ap + 64*ap + 8*ih + h] = W[h, s]; rest zero.
    w_bd = np.zeros((128, 256), dtype=np.float32)
    btn = np.zeros((8, 256), dtype=np.float32)
    for ap in range(2):
        base = 128 * ap + 64 * ap
        for ih in range(8):
            for h in range(8):
                w_bd[ih::8, base + 8 * ih + h] = W[h, :]  # rows k = 8s+ih
            btn[ih, base + 8 * ih:base + 8 * ih + 8] = -1.0
    bias = np.tile(np.asarray(b, np.float32), 16).reshape(128, 1)
    if MODE == "bf16":
        w_bd = w_bd.astype(ml_dtypes.bfloat16)
    return w_bd, btn.astype(ml_dtypes.bfloat16), bias


def kernel(stacks, mask, W, b):
    from concourse.bass_utils import run_bass_kernel_spmd

    if "nc" not in _CACHE:
        _CACHE["nc"] = _build()
    nc = _CACHE["nc"]

    stacks = np.ascontiguousarray(np.asarray(stacks, dtype=np.float32))
    mask_bf = np.ascontiguousarray(
        np.asarray(mask).astype(ml_dtypes.bfloat16))
    w_bd, btn, bias = _prep_consts(np.asarray(W, np.float32),
                                   np.asarray(b, np.float32))

    in_maps = []
    for c in range(NCORES):
        in_maps.append({
            "stacks": stacks[c * BPC:(c + 1) * BPC],
            "mask": mask_bf[c * BPC:(c + 1) * BPC],
            "w_bd": w_bd, "btn": btn, "bias": bias,
        })

    res = run_bass_kernel_spmd(nc, in_maps, core_ids=list(range(NCORES)),
                               **_CACHE.get("run_kwargs", {}))
    _CACHE["last_result"] = res
    outs = [r["out"] for r in res.results]
    return np.concatenate(outs, axis=0)


# revision 9
# speedup vs baseline: 1.2539x; 1.1215x over previous
"""Trainium2 Bass kernel for AdjStackAttentionWeights.

reference:  out = einsum('bsij,hs->bhij', stacks, W) + b[None,:,None,None]
            out = where(mask[:,None,:,:], 0.0, out)
shapes:     stacks [16,16,512,512] f32, mask [16,512,512] bool,
            W [8,16] f32, b [8] f32  ->  out [16,8,512,512] f32

Data-parallel over batch: 2 graphs per core x 8 cores.

The host shards AND re-lays-out stacks into the on-chip tile layout so
every in-DMA is a fully contiguous 4MB read (strided s-gather reads cap
at ~200GB/s on TRN2 vs ~355GB/s contiguous; same bytes either way).

Per graph, i in 4 superblocks w of 128 rows; i = 128w + 16*ih + il,
il = 8*c1 + i_in (c1 in {0,1}, i_in in [0,8)):

  rhs tile [128,8192] f32 per (b,w): p = 8s+ih, f = il*512+j
      = host-relayout srl[b,w]  (4MB contiguous DMA)
  psum [128,512] per (w,i_in): p = 8*(2ih+c1) + h = 8cd+h, j free;
      two zero-padded-lhsT matmuls accumulate (c1 = 0,1):
      lhsT w_bd[8s+ih, 128c1 + 8(2ih+c1)+h] = W[h,s]
  mask broadcast over h via tiny matmuls -> psum_k = -m
  epilogue: ACT keep=1-m ; DVE out_f = (psum + bias) * keep
  out tile [128,4096] f32 per (b,w): p = 8cd+h, f = i_in*512+j
      -> 2MB DMA (16KB runs, h-strided writes run near line rate)
Matmuls run as float32r (TF32-like) so no input cast is needed.
"""

import numpy as np
import ml_dtypes

B, S, N, H = 16, 16, 512, 8
NCORES = 8
BPC = B // NCORES  # graphs per core

MODE = "f32r"  # "f32r" | "bf16"

_CACHE = {}


def _build():
    import concourse.bacc as bacc
    import concourse.mybir as mybir
    import concourse.tile as tile

    f32 = mybir.dt.float32
    bf16 = mybir.dt.bfloat16
    cdt = mybir.dt.float32r if MODE == "f32r" else bf16

    nc = bacc.Bacc("TRN2", target_bir_lowering=False, debug=False,
                   num_devices=NCORES)

    # host-relaid stacks: [b, w, p=8s+ih, f=il*512+j]
    srl = nc.dram_tensor("srl", [BPC, 4, 128, 8192],
                         cdt if MODE == "f32r" else f32,
                         kind="ExternalInput")
    # host-relaid mask: [b, w, ih, il*512+j] bf16
    mrl = nc.dram_tensor("mrl", [BPC, 4, 8, 8192], bf16,
                         kind="ExternalInput")
    w_bd = nc.dram_tensor("w_bd", [128, 256], cdt, kind="ExternalInput")
    btn = nc.dram_tensor("btn", [8, 256], bf16, kind="ExternalInput")
    bias = nc.dram_tensor("bias", [128, 1], f32, kind="ExternalInput")
    out = nc.dram_tensor("out", [BPC, H, N, N], f32, kind="ExternalOutput")

    # out per (b, w): [cd(16), h(8), (i_in j)(4096)] ; i = 128w + 8cd + i_in
    oview = out.ap().rearrange("b h (w cd iin) j -> b w cd h (iin j)",
                               w=4, cd=16, iin=8)

    Ident = mybir.ActivationFunctionType.Identity
    ADD = mybir.AluOpType.add
    MULT = mybir.AluOpType.mult

    with tile.TileContext(nc) as tc:
        with (
            tc.tile_pool(name="const", bufs=1) as cpool,
            tc.tile_pool(name="maskp", bufs=2) as mpool,
            tc.tile_pool(name="data", bufs=3) as dpool,
            tc.tile_pool(name="keep", bufs=3) as kpool,
            tc.tile_pool(name="outp", bufs=2) as opool,
            tc.tile_pool(name="psd", bufs=3, space="PSUM") as psd_pool,
            tc.tile_pool(name="psk", bufs=3, space="PSUM") as psk_pool,
        ):
            wbd_t = cpool.tile([128, 256], cdt)
            nc.sync.dma_start(wbd_t[:], w_bd.ap())
            btn_t = cpool.tile([8, 256], bf16)
            nc.sync.dma_start(btn_t[:], btn.ap())
            bias_t = cpool.tile([128, 1], f32)
            nc.sync.dma_start(bias_t[:], bias.ap())

            for bb in range(BPC):
                for w in range(4):
                    rhs_t = dpool.tile([128, 8192], cdt, tag="rhs")
                    nc.sync.dma_start(rhs_t[:], srl.ap()[bb, w])
                    mask_t = mpool.tile([8, 8192], bf16, tag="mask")
                    nc.sync.dma_start(mask_t[:], mrl.ap()[bb, w])
                    out_t = opool.tile([128, 4096], f32)
                    for i_in in range(8):
                        ps_d = psd_pool.tile([128, 512], f32)
                        ps_k = psk_pool.tile([128, 512], f32)
                        for c1 in range(2):
                            fsl = (8 * c1 + i_in) * 512
                            nc.tensor.matmul(
                                ps_d[:, :],
                                wbd_t[:, c1 * 128:c1 * 128 + 128],
                                rhs_t[:, fsl:fsl + 512],
                                start=(c1 == 0), stop=(c1 == 1))
                            nc.tensor.matmul(
                                ps_k[:, :],
                                btn_t[:, c1 * 128:c1 * 128 + 128],
                                mask_t[:, fsl:fsl + 512],
                                start=(c1 == 0), stop=(c1 == 1))
                        keep_t = kpool.tile([128, 512], f32)
                        # keep = 1 - m   (ps_k holds -m)
                        nc.scalar.activation(keep_t[:], ps_k[:], Ident,
                                             bias=1.0, scale=1.0)
                        # out = (ps_d + bias) * keep
                        nc.vector.scalar_tensor_tensor(
                            out_t[:, i_in * 512:i_in * 512 + 512], ps_d[:],
                            bias_t[:], keep_t[:], op0=ADD, op1=MULT)
                    nc.scalar.dma_start(oview[bb, w], out_t[:])

    nc.compile()
    return nc


def _prep_consts(W, b):
    # lhsT for the c1-th accumulating matmul lives in w_bd[:, 128*c1:...]
    # w_bd[8s+ih, 128*c1 + 8*(2ih+c1) + h] = W[h, s]; rest zero.
    w_bd = np.zeros((128, 256), dtype=np.float32)
    btn = np.zeros((8, 256), dtype=np.float32)
    for c1 in range(2):
        for ih in range(8):
            base = 128 * c1 + 8 * (2 * ih + c1)
            for h in range(8):
                w_bd[ih::8, base + h] = W[h, :]  # rows k = 8s+ih
            btn[ih, base:base + 8] = -1.0
    bias = np.tile(np.asarray(b, np.float32), 16).reshape(128, 1)
    if MODE == "bf16":
        w_bd = w_bd.astype(ml_dtypes.bfloat16)
    return w_bd, btn.astype(ml_dtypes.bfloat16), bias


def _relayout(stacks, mask):
    # srl[b, w, 8s+ih, il*512+j] = stacks[b, s, 128w+16ih+il, j]
    srl = stacks.reshape(B, S, 4, 8, 16, N)          # b s w ih il j
    srl = np.ascontiguousarray(srl.transpose(0, 2, 1, 3, 4, 5))
    srl = srl.reshape(B, 4, 128, 8192)
    # mrl[b, w, ih, il*512+j] = mask[b, 128w+16ih+il, j]
    mrl = mask.reshape(B, 4, 8, 16 * N).astype(ml_dtypes.bfloat16)
    return srl, mrl


def kernel(stacks, mask, W, b):
    from concourse.bass_utils import run_bass_kernel_spmd

    if "nc" not in _CACHE:
        _CACHE["nc"] = _build()
    nc = _CACHE["nc"]

    stacks = np.asarray(stacks, dtype=np.float32)
    srl, mrl = _relayout(stacks, np.asarray(mask))
    w_bd, btn, bias = _prep_consts(np.asarray(W, np.float32),
                                   np.asarray(b, np.float32))

    in_maps = []
    for c in range(NCORES):
        in_maps.append({
            "srl": srl[c * BPC:(c + 1) * BPC],
            "mrl": mrl[c * BPC:(c + 1) * BPC],
            "w_bd": w_bd, "btn": btn, "bias": bias,
        })

    res = run_bass_kernel_spmd(nc, in_maps, core_ids=list(range(NCORES)),
                               **_CACHE.get("run_kwargs", {}))
    _CACHE["last_result"] = res
    outs = [r["out"] for r in res.results]
    return np.concatenate(outs, axis=0)


# revision 16
# speedup vs baseline: 1.6786x; 1.3388x over previous
"""Trainium2 Bass kernel for AdjStackAttentionWeights.

reference:  out = einsum('bsij,hs->bhij', stacks, W) + b[None,:,None,None]
            out = where(mask[:,None,:,:], 0.0, out)
shapes:     stacks [16,16,512,512] f32, mask [16,512,512] bool,
            W [8,16] f32, b [8] f32  ->  out [16,8,512,512] f32

Data-parallel over batch: 2 graphs per core x 8 cores.

The host shards AND re-lays-out the inputs into the exact on-chip tile
layouts so every DMA is fully contiguous (strided s-gather reads cap at
~200GB/s on TRN2 vs ~355GB/s contiguous; same HBM bytes either way).
The boolean mask is pre-broadcast over h on the host (bf16) so masking
is a plain elementwise multiply -- no broadcast matmuls on-chip.

Per graph, i in 4 superblocks w of 128 rows; i = 128w + 16*ih + il,
il = 8*c1 + i_in (c1 in {0,1}, i_in in [0,8)); cd = 2*ih + c1:

  rhs tile  [128,8192] f32 per (b,w): p = 8s+ih, f = il*512+j  (4MB DMA)
  keep tile [128,4096] bf16 per (b,w): p = 8cd+h, f = i_in*512+j (1MB DMA)
  psum [128,512] per (w,i_in): p = 8cd+h; two zero-padded-lhsT matmuls
      accumulate (c1=0,1): lhsT w_bd[8s+ih, 128c1 + 8(2ih+c1)+h] = W[h,s]
  epilogue: one DVE op: out = (psum + bias) * keep
  out tile [128,4096] f32 per (b,w): p = 8cd+h, f = i_in*512+j (2MB DMA)
Matmuls run as float32r so no input cast is needed.
"""

import numpy as np
import ml_dtypes

B, S, N, H = 16, 16, 512, 8
NCORES = 8
BPC = B // NCORES  # graphs per core

MODE = "f32r"  # "f32r" | "bf16"

_CACHE = {}


def _build():
    import concourse.bacc as bacc
    import concourse.mybir as mybir
    import concourse.tile as tile

    f32 = mybir.dt.float32
    bf16 = mybir.dt.bfloat16
    cdt = mybir.dt.float32r if MODE == "f32r" else bf16

    nc = bacc.Bacc("TRN2", target_bir_lowering=False, debug=False,
                   num_devices=NCORES)

    # host-relaid stacks: [b, w, p=8s+ih, f=il*512+j]
    srl = nc.dram_tensor("srl", [BPC, 4, 128, 8192],
                         cdt if MODE == "f32r" else f32,
                         kind="ExternalInput")
    # host-broadcast keep mask: [b, w, p=8cd+h, f=i_in*512+j] bf16
    krl = nc.dram_tensor("krl", [BPC, 4, 128, 4096], bf16,
                         kind="ExternalInput")
    w_bd = nc.dram_tensor("w_bd", [128, 256], cdt, kind="ExternalInput")
    bias = nc.dram_tensor("bias", [128, 1], f32, kind="ExternalInput")
    out = nc.dram_tensor("out", [BPC, H, N, N], f32, kind="ExternalOutput")

    # out per (b, w): [cd(16), h(8), (i_in j)(4096)] ; i = 128w + 8cd + i_in
    oview = out.ap().rearrange("b h (w cd iin) j -> b w cd h (iin j)",
                               w=4, cd=16, iin=8)

    ADD = mybir.AluOpType.add
    MULT = mybir.AluOpType.mult

    with tile.TileContext(nc) as tc:
        with (
            tc.tile_pool(name="const", bufs=1) as cpool,
            tc.tile_pool(name="maskp", bufs=2) as mpool,
            tc.tile_pool(name="data", bufs=3) as dpool,
            tc.tile_pool(name="outp", bufs=2) as opool,
            tc.tile_pool(name="psd", bufs=6, space="PSUM") as psd_pool,
        ):
            wbd_t = cpool.tile([128, 256], cdt)
            nc.sync.dma_start(wbd_t[:], w_bd.ap())
            bias_t = cpool.tile([128, 1], f32)
            nc.sync.dma_start(bias_t[:], bias.ap())

            for bb in range(BPC):
                for w in range(4):
                    rhs_t = dpool.tile([128, 8192], cdt, tag="rhs")
                    nc.sync.dma_start(rhs_t[:], srl.ap()[bb, w])
                    mask_t = mpool.tile([128, 4096], bf16, tag="mask")
                    nc.sync.dma_start(mask_t[:], krl.ap()[bb, w])
                    out_t = opool.tile([128, 4096], f32)
                    for i_in in range(8):
                        ps_d = psd_pool.tile([128, 512], f32)
                        for c1 in range(2):
                            fsl = (8 * c1 + i_in) * 512
                            nc.tensor.matmul(
                                ps_d[:, :],
                                wbd_t[:, c1 * 128:c1 * 128 + 128],
                                rhs_t[:, fsl:fsl + 512],
                                start=(c1 == 0), stop=(c1 == 1))
                        # out = (ps_d + bias) * keep
                        nc.vector.scalar_tensor_tensor(
                            out_t[:, i_in * 512:i_in * 512 + 512], ps_d[:],
                            bias_t[:],
                            mask_t[:, i_in * 512:i_in * 512 + 512],
                            op0=ADD, op1=MULT)
                    nc.scalar.dma_start(oview[bb, w], out_t[:])

    nc.compile()
    return nc


def _prep_consts(W, b):
    # lhsT for the c1-th accumulating matmul lives in w_bd[:, 128*c1:...]
    # w_bd[8s+ih, 128*c1 + 8*(2ih+c1) + h] = W[h, s]; rest zero.
    w_bd = np.zeros((128, 256), dtype=np.float32)
    for c1 in range(2):
        for ih in range(8):
            base = 128 * c1 + 8 * (2 * ih + c1)
            for h in range(8):
                w_bd[ih::8, base + h] = W[h, :]  # rows k = 8s+ih
    bias = np.tile(np.asarray(b, np.float32), 16).reshape(128, 1)
    if MODE == "bf16":
        w_bd = w_bd.astype(ml_dtypes.bfloat16)
    return w_bd, bias


def _relayout(stacks, mask):
    # srl[b, w, 8s+ih, il*512+j] = stacks[b, s, 128w+16ih+il, j]
    srl = stacks.reshape(B, S, 4, 8, 16, N)          # b s w ih il j
    srl = np.ascontiguousarray(srl.transpose(0, 2, 1, 3, 4, 5))
    srl = srl.reshape(B, 4, 128, 8192)
    # krl[b, w, 8cd+h, i_in*512+j] = 1 - mask[b, 128w+8cd+i_in, j]
    keep = (~np.asarray(mask, bool)).reshape(B, 4, 16, 8, N)  # b w cd iin j
    krl = np.broadcast_to(keep[:, :, :, None, :, :],
                          (B, 4, 16, 8, 8, N))                # b w cd h iin j
    krl = np.ascontiguousarray(krl.astype(ml_dtypes.bfloat16))
    krl = krl.reshape(B, 4, 128, 4096)
    return srl, krl


def kernel(stacks, mask, W, b):
    from concourse.bass_utils import run_bass_kernel_spmd

    if "nc" not in _CACHE:
        _CACHE["nc"] = _build()
    nc = _CACHE["nc"]

    stacks = np.asarray(stacks, dtype=np.float32)
    srl, krl = _relayout(stacks, np.asarray(mask))
    w_bd, bias = _prep_consts(np.asarray(W, np.float32),
                              np.asarray(b, np.float32))

    in_maps = []
    for c in range(NCORES):
        in_maps.append({
            "srl": srl[c * BPC:(c + 1) * BPC],
            "krl": krl[c * BPC:(c + 1) * BPC],
            "w_bd": w_bd, "bias": bias,
        })

    res = run_bass_kernel_spmd(nc, in_maps, core_ids=list(range(NCORES)),
                               **_CACHE.get("run_kwargs", {}))
    _CACHE["last_result"] = res
    outs = [r["out"] for r in res.results]
    return np.concatenate(outs, axis=0)
